# revision 26
# baseline (speedup 1.0000x reference)
"""HGT layer kernel for 8 Trainium2 NeuronCores.

The axon-tunneled setup makes host<->device transfer (~35 MB/s each way,
full duplex) the overwhelming bottleneck; device exec is a few ms. So the
design minimizes wire bytes end to end:
  - Each core owns N/8=2500 destination nodes and their incoming edges.
  - Uploads per core (~0.95 MB): fp8(e4m3) h rows (scores/values tolerate
    the ~3% noise; h is range-scaled by s8 when needed and W{q,k,v} absorb
    1/s8), int16 gather indices uploaded at 16 partitions and replicated to
    128 on device, uint8 one-hot column ids, and a 1/8 shard of the packed
    weights (AllGathered on device instead of uploading 8 copies).
  - Device: PE-transpose h, project q/kv (fp16), AllGather the kv table,
    then per window of <=128 dst nodes (2048 edge slots): dma_gather kv[src]
    and q[dst] rows, DVE dot-product scores, ACT exp, build the dst one-hot
    on device with iota+is_equal, PE onehot-matmul aggregation of
    [messages | exp] into PSUM, normalize, flush.
  - Final: transpose-gather normalized agg -> output projection; the device
    returns only OSC*alpha*trans as fp8 (scaled into e4m3's normal range);
    the skip term (1-alpha)*h is added on the host from the fp32 h.
  - Download: fp8 out fetched per-shard with 8 threads.
  - Transfer/compute overlap: the h8 and weight uploads are dispatched
    asynchronously at function entry so they stream over the tunnel while
    the host does edge preprocessing; the index uploads are dispatched
    before the host computes the skip term. The timed device roundtrip is
    then dominated by the (irreducible) fp8 output download.
  - The jitted PJRT executable is cached in module globals so repeat calls
    pay no retrace/recompile; donated output buffers are created on device.
"""

import math
import concurrent.futures as _cf
import numpy as np

import jax
import jax.numpy as jnp
from jax.experimental.shard_map import shard_map
from jax.sharding import Mesh, NamedSharding, PartitionSpec as P

import concourse.bacc as bacc
import concourse.tile as tile
import concourse.bass as bass
from concourse import mybir
from concourse.bass2jax import (
    _bass_exec_p,
    install_neuronx_cc_hook,
    partition_id_tensor,
)

N = 20000
E = 320000
D = 256
H = 8
DK = 32
NCORES = 8
NPC = N // NCORES          # 2500 nodes per core
NTN = 2560                 # padded nodes per core (20 tiles of 128)
NTILES = NTN // 128        # 20
WSLOTS = 2048              # edge slots per window
WCH = WSLOTS // 128        # 16 chunks per window
WSPAN = 128                # max dst nodes per window

F16 = mybir.dt.float16
F8 = mybir.dt.float8e4
U8 = mybir.dt.uint8
F32 = mybir.dt.float32
I16 = mybir.dt.int16

_cache = {}
_MESH = None
LAST_RESULTS = None
LAST_EXEC_NS = None
LAST_INMAPS = None


def _mesh_sh():
    global _MESH
    if _MESH is None:
        devices = jax.devices()[:NCORES]
        mesh = Mesh(np.asarray(devices), ("core",))
        _MESH = (mesh, NamedSharding(mesh, P("core")))
    return _MESH


def _build(NW, use_bias, use_fbias):
    IDXL = 2 * NW * 128 + NTN // 16  # packed idx columns (sidx | qidx | vidx)
    nc = bacc.Bacc()
    h8 = nc.declare_dram_parameter("h8", [NPC, D], F8, isOutput=False)
    wsh = nc.declare_dram_parameter("wsh", [2, 128, 128], F16, isOutput=False)
    idxp = nc.declare_dram_parameter("idxp", [16, IDXL], I16, isOutput=False)
    colx = nc.declare_dram_parameter("colx", [128, NW * WCH], U8, isOutput=False)
    if use_bias:
        bqkv = nc.declare_dram_parameter("bqkv", [1, 3 * D], F16, isOutput=False)
    if use_fbias:
        bfin = nc.declare_dram_parameter("bfin", [1, D], F16, isOutput=False)
    outp = nc.declare_dram_parameter("out", [NPC, D], F8, isOutput=True)

    with tile.TileContext(nc) as tc:
        with (
            tc.tile_pool(name="const", bufs=1) as constp,
            tc.tile_pool(name="dram", bufs=1, space="DRAM") as dram,
            tc.tile_pool(name="proj", bufs=3) as projp,
            tc.tile_pool(name="psum", bufs=2, space="PSUM") as psump,
            tc.tile_pool(name="edge", bufs=2) as edgep,
            tc.tile_pool(name="fin", bufs=2) as finp,
        ):
            q_tab = dram.tile([NTN, D], F16)
            kv_slice = dram.tile([NTN, 2 * D], F16)
            kv_full = nc.dram_tensor(
                "kv_full", [NCORES * NTN, 2 * D], F16, addr_space="Shared")
            w_all = nc.dram_tensor(
                "w_all", [NCORES, 2, 128, 128], F16, addr_space="Shared")
            vn = dram.tile([NW * 128, D], F16)

            # ---- weights: AllGather the 8 shards, then lay out in SBUF ----
            wstage = dram.tile([2, 128, 128], F16)
            nc.sync.dma_start(wstage[:], wsh[:])
            nc.gpsimd.collective_compute(
                "AllGather",
                mybir.AluOpType.bypass,
                replica_groups=[list(range(NCORES))],
                ins=[wstage.opt()],
                outs=[w_all[:]],
            )
            # wpack_sb[p, j, c*128 + q] = w_all[c, j, p, q]
            wpack_sb = constp.tile([128, 2, NCORES * 128], F16)
            for j in (0, 1):
                nc.sync.dma_start(
                    wpack_sb[:, j, :].rearrange("p (c q) -> p c q", c=NCORES),
                    w_all[:, j].rearrange("c p q -> p c q"))
            wq_sb = wpack_sb[:, :, 0:D]
            wkv_sb = wpack_sb[:, :, D:3 * D]
            wa_sb = wpack_sb[:, :, 3 * D:4 * D]

            # ---- constants ----
            NTF = NPC // 128           # 19 full tiles
            NTAIL = NPC - NTF * 128    # 68 tail rows
            h8_sb = constp.tile([128, NTILES, D], F8)
            nc.vector.memset(h8_sb[:, NTF, :], 0.0)
            nc.sync.dma_start(
                h8_sb[:, 0:NTF, :],
                h8[0:NTF * 128].rearrange("(t p) d -> p t d", p=128))
            nc.sync.dma_start(h8_sb[0:NTAIL, NTF, :], h8[NTF * 128:NPC])
            h_sb = constp.tile([128, NTILES, D], F16)
            nc.vector.tensor_copy(h_sb[:], h8_sb[:])
            idx_sb = constp.tile([128, IDXL], I16)
            for g in range(8):
                nc.sync.dma_start(idx_sb[g * 16:(g + 1) * 16, :], idxp[:])
            sidx_sb = idx_sb[:, 0:NW * 128]
            qidx_sb = idx_sb[:, NW * 128:2 * NW * 128]
            vidx_sb = idx_sb[:, 2 * NW * 128:IDXL]
            colx_sb = constp.tile([128, NW * WCH], U8)
            nc.sync.dma_start(colx_sb[:], colx[:])
            iota_sb = constp.tile([128, WCH, 128], U8)
            nc.gpsimd.iota(
                iota_sb[:], [[0, WCH], [1, 128]], channel_multiplier=0,
                allow_small_or_imprecise_dtypes=True,
            )
            ident = constp.tile([128, 128], F16)
            nc.vector.memset(ident[:], 0.0)
            nc.gpsimd.affine_select(
                out=ident[:], in_=ident[:],
                compare_op=mybir.AluOpType.not_equal, fill=1.0,
                base=0, pattern=[[-1, 128]], channel_multiplier=1,
            )
            if use_bias or use_fbias:
                ones_sb = constp.tile([1, 128], F16)
                nc.vector.memset(ones_sb[:], 1.0)
            if use_bias:
                bqkv_sb = constp.tile([1, 3 * D], F16)
                nc.sync.dma_start(bqkv_sb[:], bqkv[:])
            if use_fbias:
                bfin_sb = constp.tile([1, D], F16)
                nc.sync.dma_start(bfin_sb[:], bfin[:])

            # ---- transpose h: hT_sb[:, j, node] = h[node, j*128+p] ----
            hT_sb = constp.tile([128, 2, NTN], F16)
            for nt in range(NTILES):
                for j in (0, 1):
                    pt = psump.tile([128, 128], F16, tag="pt")
                    nc.tensor.transpose(
                        pt[:], h_sb[:, nt, j * 128:(j + 1) * 128], ident[:])
                    nc.vector.tensor_copy(
                        hT_sb[:, j, nt * 128:(nt + 1) * 128], pt[:])

            # ---- projection phase ----
            for nt in range(NTILES):
                sl = slice(nt * 128, (nt + 1) * 128)
                pkv = psump.tile([128, 2 * D], F32, tag="pkv")
                for j in (0, 1):
                    nc.tensor.matmul(
                        pkv[:], hT_sb[:, j, sl], wkv_sb[:, j, :],
                        start=(j == 0), stop=(j == 1 and not use_bias),
                    )
                if use_bias:
                    nc.tensor.matmul(
                        pkv[:], ones_sb[:], bqkv_sb[:, D:3 * D],
                        start=False, stop=True)
                kv_sb = projp.tile([128, 2 * D], F16, tag="kv")
                nc.vector.tensor_copy(kv_sb[:], pkv[:])
                nc.sync.dma_start(kv_slice[sl, :], kv_sb[:])

                pq = psump.tile([128, D], F32, tag="pq")
                for j in (0, 1):
                    nc.tensor.matmul(
                        pq[:], hT_sb[:, j, sl], wq_sb[:, j, :],
                        start=(j == 0), stop=(j == 1 and not use_bias),
                    )
                if use_bias:
                    nc.tensor.matmul(
                        pq[:], ones_sb[:], bqkv_sb[:, 0:D],
                        start=False, stop=True)
                q_sb = projp.tile([128, D], F16, tag="q")
                nc.vector.tensor_copy(q_sb[:], pq[:])
                nc.sync.dma_start(q_tab[sl, :], q_sb[:])

            nc.gpsimd.collective_compute(
                "AllGather",
                mybir.AluOpType.bypass,
                replica_groups=[list(range(NCORES))],
                ins=[kv_slice.opt()],
                outs=[kv_full[:]],
            )

            # ---- edge phase ----
            for w in range(NW):
                csl = slice(w * 128, (w + 1) * 128)
                kvg = edgep.tile([128, WCH, 2 * D], F16, tag="kvg")
                nc.gpsimd.dma_gather(
                    kvg[:], kv_full[:], sidx_sb[:, csl],
                    num_idxs=WSLOTS, num_idxs_reg=WSLOTS, elem_size=2 * D,
                    single_packet=False,
                )
                qg = edgep.tile([128, WCH, D], F16, tag="qg")
                nc.gpsimd.dma_gather(
                    qg[:], q_tab[:], qidx_sb[:, csl],
                    num_idxs=WSLOTS, num_idxs_reg=WSLOTS, elem_size=D,
                    single_packet=False,
                )
                oa_sb = edgep.tile([128, WCH, 128], F16, tag="oa")
                nc.vector.tensor_tensor(
                    oa_sb[:],
                    colx_sb[:, w * WCH:(w + 1) * WCH].broadcast_to([128, WCH, 128]),
                    iota_sb[:],
                    mybir.AluOpType.is_equal,
                )

                prod = edgep.tile([128, WCH, D], F16, tag="prod")
                nc.vector.tensor_mul(prod[:], qg[:], kvg[:, :, 0:D])
                scores = edgep.tile([128, WCH, H], F32, tag="sc")
                nc.vector.tensor_reduce(
                    scores[:],
                    prod[:].rearrange("p c (h k) -> p c h k", h=H),
                    axis=mybir.AxisListType.X,
                    op=mybir.AluOpType.add,
                )
                msgz = edgep.tile([128, WCH, D + H], F16, tag="msgz")
                nc.scalar.activation(
                    msgz[:, :, D:D + H], scores[:], mybir.ActivationFunctionType.Exp
                )
                nc.vector.tensor_mul(
                    msgz[:, :, 0:D].rearrange("p c (h k) -> p c h k", h=H),
                    kvg[:, :, D:2 * D].rearrange("p c (h k) -> p c h k", h=H),
                    msgz[:, :, D:D + H].broadcast_to([128, WCH, H, DK]),
                )
                pw = psump.tile([128, D + H], F32, tag="pkv")
                for i in range(WCH):
                    nc.tensor.matmul(
                        pw[:], oa_sb[:, i, :], msgz[:, i, :],
                        start=(i == 0), stop=(i == WCH - 1),
                    )
                zr = finp.tile([128, H], F32, tag="zr")
                nc.vector.tensor_scalar_add(zr[:], pw[:, D:D + H], 1e-30)
                zrec = finp.tile([128, H], F32, tag="zrec")
                nc.vector.reciprocal(zrec[:], zr[:])
                vb = finp.tile([128, D], F16, tag="vb")
                nc.vector.tensor_mul(
                    vb[:].rearrange("p (h k) -> p h k", h=H),
                    pw[:, 0:D].rearrange("p (h k) -> p h k", h=H),
                    zrec[:].broadcast_to([128, H, DK]),
                )
                nc.sync.dma_start(vn[csl, :], vb[:])

            # ---- final phase ----
            tg = constp.tile([128, 2, NTN], F16)
            nc.gpsimd.dma_gather(
                tg[:], vn[:], vidx_sb[:],
                num_idxs=NTN, num_idxs_reg=NTN, elem_size=D, transpose=True,
                single_packet=False,
            )
            for nt in range(NTILES):
                sl = slice(nt * 128, (nt + 1) * 128)
                po = psump.tile([128, D], F32, tag="pq")
                for j in (0, 1):
                    nc.tensor.matmul(
                        po[:], tg[:, j, sl], wa_sb[:, j, :],
                        start=(j == 0), stop=(j == 1 and not use_fbias),
                    )
                if use_fbias:
                    nc.tensor.matmul(
                        po[:], ones_sb[:], bfin_sb[:], start=False, stop=True)
                ot = finp.tile([128, D], F8, tag="ot")
                nc.vector.tensor_copy(ot[:], po[:])
                if (nt + 1) * 128 <= NPC:
                    nc.sync.dma_start(outp[sl, :], ot[:])
                elif nt * 128 < NPC:
                    nc.sync.dma_start(outp[nt * 128:NPC, :], ot[0:NPC - nt * 128, :])

    nc.compile()
    return nc


def _make_runner(nc):
    install_neuronx_cc_hook()
    partition_name = nc.partition_id_tensor.name if nc.partition_id_tensor else None
    in_names, out_names, out_avals = [], [], []
    for alloc in nc.m.functions[0].allocations:
        if not isinstance(alloc, mybir.MemoryLocationSet):
            continue
        name = alloc.memorylocations[0].name
        if alloc.kind == "ExternalInput":
            if name != partition_name:
                in_names.append(name)
        elif alloc.kind == "ExternalOutput":
            out_names.append(name)
            out_avals.append(jax.core.ShapedArray(
                tuple(alloc.tensor_shape), mybir.dt.np(alloc.dtype)))
    n_params = len(in_names)
    bind_names = in_names + out_names
    if partition_name is not None:
        bind_names = bind_names + [partition_name]
    donate = tuple(range(n_params, n_params + len(out_names)))

    def _body(*args):
        operands = list(args)
        if partition_name is not None:
            operands.append(partition_id_tensor())
        outs = _bass_exec_p.bind(
            *operands,
            out_avals=tuple(out_avals),
            in_names=tuple(bind_names),
            out_names=tuple(out_names),
            lowering_input_output_aliases=(),
            sim_require_finite=True,
            sim_require_nnan=True,
            nc=nc,
        )
        return tuple(outs)

    mesh, zsh = _mesh_sh()
    in_specs = (P("core"),) * (n_params + len(out_names))
    out_specs = (P("core"),) * len(out_names)
    fn = jax.jit(
        shard_map(_body, mesh=mesh, in_specs=in_specs, out_specs=out_specs,
                  check_rep=False),
        donate_argnums=donate, keep_unused=True,
    )
    zeros_fn = jax.jit(
        lambda: tuple(
            jnp.zeros((NCORES * a.shape[0], *a.shape[1:]), a.dtype)
            for a in out_avals),
        out_shardings=(zsh,) * len(out_names) if len(out_names) > 1 else zsh,
    )

    import os, time as _t
    dbg = bool(os.environ.get("KERNEL_TIMING"))
    pool = _cf.ThreadPoolExecutor(NCORES)

    def run(globals_map, zeros=None):
        t0 = _t.perf_counter()
        args = [globals_map[name] for name in in_names]
        t1 = _t.perf_counter()
        if zeros is None:
            zeros = zeros_fn()
        if len(out_names) == 1 and not isinstance(zeros, tuple):
            zeros = (zeros,)
        t2 = _t.perf_counter()
        out_arrs = fn(*args, *zeros)
        t3 = _t.perf_counter()
        results = {}
        for i, name in enumerate(out_names):
            shards = sorted(out_arrs[i].addressable_shards,
                            key=lambda s: s.device.id)
            datas = [s.data for s in shards]
            for d in datas:
                try:
                    d.copy_to_host_async()
                except AttributeError:
                    break
            parts = list(pool.map(np.asarray, datas))
            results[name] = parts
        t4 = _t.perf_counter()
        if dbg:
            print(f"[run] gather_args={t1-t0:.3f} zeros={t2-t1:.3f} "
                  f"dispatch={t3-t2:.3f} fetch={t4-t3:.3f}", flush=True)
        return results

    run.fn = fn
    run.zeros_fn = zeros_fn
    run.in_names = in_names
    run.out_names = out_names
    return run


def _wrap16(v):
    """[L] int array -> [16, L//16] wrapped int16: tile[p, s] = v[s*16+p]."""
    L = v.shape[0]
    return np.ascontiguousarray(v.reshape(L // 16, 16).T.astype(np.int16))


def _wrap16_win(v):
    """[NW, WSLOTS] -> [16, NW*128]: per-window wrapped layout."""
    NW = v.shape[0]
    w = v.reshape(NW, WSLOTS // 16, 16).transpose(2, 0, 1)
    return np.ascontiguousarray(w.reshape(16, NW * (WSLOTS // 16)).astype(np.int16))


def kernel(h, src, dst, Wk, bk, Wq, bq, Wv, bv, Wa, ba, rel_att, rel_msg, rel_pri, skip):
    global LAST_RESULTS, LAST_EXEC_NS
    h = np.asarray(h, np.float32)
    src = np.asarray(src, np.int32)
    dst = np.asarray(dst, np.int32)

    # ---- fold weights on host ----
    scale = (np.asarray(rel_pri, np.float32) / math.sqrt(DK)).astype(np.float32)
    WqT = np.asarray(Wq, np.float32).T.reshape(D, H, DK)
    Wq_eff = (WqT * scale[None, :, None]).reshape(D, D)
    bq_eff = (np.asarray(bq, np.float32).reshape(H, DK) * scale[:, None]).reshape(D)
    WkT = np.asarray(Wk, np.float32).T.reshape(D, H, DK)
    Wk_eff = np.einsum("dhk,hke->dhe", WkT, np.asarray(rel_att, np.float32)).reshape(D, D)
    bk_eff = np.einsum("hk,hke->he", np.asarray(bk, np.float32).reshape(H, DK),
                       np.asarray(rel_att, np.float32)).reshape(D)
    WvT = np.asarray(Wv, np.float32).T.reshape(D, H, DK)
    Wv_eff = np.einsum("dhk,hke->dhe", WvT, np.asarray(rel_msg, np.float32)).reshape(D, D)
    bv_eff = np.einsum("hk,hke->he", np.asarray(bv, np.float32).reshape(H, DK),
                       np.asarray(rel_msg, np.float32)).reshape(D)
    alpha = float(1.0 / (1.0 + math.exp(-float(np.asarray(skip)))))
    # h is uploaded fp8, pre-scaled by s8 to fit e4m3 range; W{q,k,v} absorb
    # 1/s8 (scores and v are then exact w.r.t. the scaling). The skip term
    # (1-alpha)*h is added on the host from the fp32 h, so the device output
    # is just alpha*trans, scaled by OSC to sit in e4m3's normal range.
    hmax = float(np.abs(h).max()) if h.size else 1.0
    s8 = 1.0 if hmax <= 192.0 else 128.0 / hmax
    OSC = 32.0
    Wq_dev = Wq_eff / s8
    Wkv_dev = np.concatenate([Wk_eff, Wv_eff], axis=1) / s8   # [256, 512]
    bkv_eff = np.concatenate([bk_eff, bv_eff])                # [512]
    Wa_dev = (OSC * alpha * np.asarray(Wa, np.float32).T)     # [256, 256]
    ba_fin = OSC * alpha * np.asarray(ba, np.float32)
    use_bias = bool(np.any(bq_eff) or np.any(bkv_eff))
    use_fbias = bool(np.any(ba_fin))

    # ---- start the big uploads NOW: h8 (2/3 of the upload bytes) and the
    # weight shards stream over the tunnel asynchronously while the host
    # does the edge preprocessing below (transfer/compute overlap) ----
    f16 = np.float16
    f8np = mybir.dt.np(F8)
    _, csh = _mesh_sh()
    h8_g = jax.device_put(np.ascontiguousarray((s8 * h).astype(f8np)), csh)
    wpack = np.concatenate([Wq_dev, Wkv_dev, Wa_dev], axis=1)  # [256, 1024]
    wpack = np.ascontiguousarray(
        wpack.reshape(2, 128, NCORES, 128).transpose(2, 0, 1, 3)
        .astype(f16).reshape(NCORES * 2, 128, 128))
    wsh_g = jax.device_put(wpack, csh)

    # ---- edge preprocessing ----
    order = np.argsort(dst, kind="stable")
    dsts = dst[order]
    srcs = src[order]
    core_of = dsts // NPC
    core_starts = np.searchsorted(core_of, np.arange(NCORES + 1))
    deg = np.bincount(dst, minlength=N)

    # window packing per core: LPT bin packing of nodes into windows of
    # <=WSPAN nodes and <=WSLOTS edge slots (largest-degree first into the
    # least-loaded window). Windows are arbitrary node sets; each node gets
    # (win_id, col) used for the one-hot column, q-row index, and its row
    # w*128+col in the vn buffer. Retries with one more window on overflow.
    def _lpt_pack(degs, nb):
        order_d = np.argsort(-degs, kind="stable")
        used = np.zeros(nb, np.int64)
        cnt = np.zeros(nb, np.int64)
        win_id = np.empty(NPC, np.int64)
        col_of = np.empty(NPC, np.int64)
        for node in order_d:
            d = int(degs[node])
            cand = np.where(cnt < WSPAN)[0]
            b = cand[np.argmin(used[cand])]
            if used[b] + d > WSLOTS:
                return None
            win_id[node] = b
            col_of[node] = cnt[b]
            used[b] += d
            cnt[b] += 1
        return win_id, col_of

    core_meta = []  # per core: (win_id[NPC], col_of[NPC], n_windows)
    NW_max = 0
    for c in range(NCORES):
        degs = deg[c * NPC:(c + 1) * NPC]
        assert degs.max(initial=0) <= WSLOTS, "node degree exceeds window capacity"
        nb = max(int(np.ceil(degs.sum() / WSLOTS)),
                 int(np.ceil(NPC / WSPAN)))
        while True:
            r = _lpt_pack(degs, nb)
            if r is not None:
                break
            nb += 1
        core_meta.append((r[0], r[1], nb))
        NW_max = max(NW_max, nb)
    NW = NW_max

    key = (NW, use_bias, use_fbias)
    if key not in _cache:
        nc = _build(NW, use_bias, use_fbias)
        _cache[key] = (nc, _make_runner(nc))
    nc, run = _cache[key]

    # ---- per-core index tables ----
    idxp_parts = []
    colx_parts = []
    if use_bias:
        bqkv_in = np.concatenate([bq_eff, bkv_eff]).reshape(1, 3 * D).astype(f16)
    if use_fbias:
        bfin_in = ba_fin.reshape(1, D).astype(f16)

    for c in range(NCORES):
        n0 = c * NPC
        e0, e1 = core_starts[c], core_starts[c + 1]
        ed = dsts[e0:e1] - n0         # local dst
        es = srcs[e0:e1]              # global src
        win_id, col_of, _nw = core_meta[c]
        wid = win_id[ed]
        # slot assignment: within window, sort edges by src kv row for locality
        es_row = (es // NPC) * NTN + (es % NPC)
        sort2 = np.lexsort((es_row, wid))
        ed = ed[sort2]
        es_row = es_row[sort2]
        wid = wid[sort2]
        # rank within window
        wcounts = np.bincount(wid, minlength=NW)
        woff = np.zeros(NW + 1, np.int64)
        np.cumsum(wcounts, out=woff[1:])
        rank = np.arange(e1 - e0) - woff[wid]
        slot = wid * WSLOTS + rank    # global slot in [0, NW*WSLOTS)

        src_slots = np.zeros((NW, WSLOTS), np.int64)
        q_slots = np.zeros((NW, WSLOTS), np.int64)
        src_slots.reshape(-1)[slot] = es_row
        q_slots.reshape(-1)[slot] = ed
        # one-hot column per slot: [128, NW*WCH]; slot s=(w, c*128+p) ->
        # colx[p, w*WCH+c]; 255 for empty slots (matches no iota value)
        col_slots = np.full((NW, WSLOTS), 255, np.int64)
        col_slots.reshape(-1)[slot] = col_of[ed]
        colx_np = np.ascontiguousarray(
            col_slots.reshape(NW, WCH, 128).transpose(2, 0, 1)
            .reshape(128, NW * WCH).astype(np.uint8))

        # vrow: local node -> its row in vn
        vrow = np.zeros(NTN, np.int64)
        vrow[:NPC] = win_id * 128 + col_of

        idxp_parts.append(np.concatenate(
            [_wrap16_win(src_slots), _wrap16_win(q_slots), _wrap16(vrow)], axis=1))
        colx_parts.append(colx_np)

    globals_map = {
        "h8": h8_g,
        "wsh": wsh_g,
        "idxp": jax.device_put(np.concatenate(idxp_parts, axis=0), csh),
        "colx": jax.device_put(np.concatenate(colx_parts, axis=0), csh),
    }
    if use_bias:
        globals_map["bqkv"] = jax.device_put(
            np.concatenate([bqkv_in] * NCORES, axis=0), csh)
    if use_fbias:
        globals_map["bfin"] = jax.device_put(
            np.concatenate([bfin_in] * NCORES, axis=0), csh)
    # donated output buffers materialize on device and the host skip term
    # computes while the index tables stream to the devices
    zeros = run.zeros_fn()
    skip_part = (1.0 - alpha) * h

    global LAST_INMAPS
    LAST_INMAPS = globals_map
    import time as _time
    _t0 = _time.perf_counter()
    res = run(globals_map, zeros)
    LAST_RESULTS = res
    LAST_EXEC_NS = int((_time.perf_counter() - _t0) * 1e9)

    out = np.concatenate(
        [res["out"][c] for c in range(NCORES)], axis=0).astype(np.float32)
    out *= 1.0 / OSC
    out += skip_part
    return out


# revision 27
# speedup vs baseline: 1.0656x; 1.0656x over previous
"""HGT layer kernel for 8 Trainium2 NeuronCores.

The axon-tunneled setup makes host<->device transfer (~35 MB/s each way,
full duplex) the overwhelming bottleneck; device exec is a few ms. So the
design minimizes wire bytes end to end:
  - Each core owns N/8=2500 destination nodes and their incoming edges.
  - Uploads per core (~0.95 MB): fp8(e4m3) h rows (scores/values tolerate
    the ~3% noise; h is range-scaled by s8 when needed and W{q,k,v} absorb
    1/s8), int16 gather indices uploaded at 16 partitions and replicated to
    128 on device, uint8 one-hot column ids, and a 1/8 shard of the packed
    weights (AllGathered on device instead of uploading 8 copies).
  - Device: PE-transpose h, project q/kv (fp16), AllGather the kv table,
    then per window of <=128 dst nodes (2048 edge slots): dma_gather kv[src]
    and q[dst] rows, DVE dot-product scores, ACT exp, build the dst one-hot
    on device with iota+is_equal, PE onehot-matmul aggregation of
    [messages | exp] into PSUM, normalize, flush.
  - Final: transpose-gather normalized agg -> output projection; the device
    returns only OSC*alpha*trans as fp8 (scaled into e4m3's normal range);
    the skip term (1-alpha)*h is added on the host from the fp32 h.
  - Download: fp8 out fetched per-shard with 8 threads.
  - Transfer/compute overlap: the h8 and weight uploads are dispatched
    asynchronously at function entry so they stream over the tunnel while
    the host does edge preprocessing; the index uploads are dispatched
    before the host computes the skip term. The timed device roundtrip is
    then dominated by the (irreducible) fp8 output download.
  - The jitted PJRT executable is cached in module globals so repeat calls
    pay no retrace/recompile; donated output buffers are created on device.
"""

import math
import concurrent.futures as _cf
import numpy as np

import jax
import jax.numpy as jnp
from jax.experimental.shard_map import shard_map
from jax.sharding import Mesh, NamedSharding, PartitionSpec as P

import concourse.bacc as bacc
import concourse.tile as tile
import concourse.bass as bass
from concourse import mybir
from concourse.bass2jax import (
    _bass_exec_p,
    install_neuronx_cc_hook,
    partition_id_tensor,
)

N = 20000
E = 320000
D = 256
H = 8
DK = 32
NCORES = 8
NPC = N // NCORES          # 2500 nodes per core
NTN = 2560                 # padded nodes per core (20 tiles of 128)
NTILES = NTN // 128        # 20
WSLOTS = 2048              # edge slots per window
WCH = WSLOTS // 128        # 16 chunks per window
WSPAN = 128                # max dst nodes per window

F16 = mybir.dt.float16
F8 = mybir.dt.float8e4
U8 = mybir.dt.uint8
F32 = mybir.dt.float32
I16 = mybir.dt.int16

_cache = {}
_MESH = None
LAST_RESULTS = None
LAST_EXEC_NS = None
LAST_INMAPS = None


def _mesh_sh():
    global _MESH
    if _MESH is None:
        devices = jax.devices()[:NCORES]
        mesh = Mesh(np.asarray(devices), ("core",))
        _MESH = (mesh, NamedSharding(mesh, P("core")))
    return _MESH


def _build(NW, use_bias, use_fbias):
    IDXL = 2 * NW * 128 + NTN // 16  # packed idx columns (sidx | qidx | vidx)
    nc = bacc.Bacc()
    h8 = nc.declare_dram_parameter("h8", [NPC, D], F8, isOutput=False)
    wsh = nc.declare_dram_parameter("wsh", [2, 128, 128], F16, isOutput=False)
    idxp = nc.declare_dram_parameter("idxp", [16, IDXL], I16, isOutput=False)
    colx = nc.declare_dram_parameter("colx", [128, NW * WCH], U8, isOutput=False)
    if use_bias:
        bqkv = nc.declare_dram_parameter("bqkv", [1, 3 * D], F16, isOutput=False)
    if use_fbias:
        bfin = nc.declare_dram_parameter("bfin", [1, D], F16, isOutput=False)
    outp = nc.declare_dram_parameter("out", [NPC, D], F8, isOutput=True)

    with tile.TileContext(nc) as tc:
        with (
            tc.tile_pool(name="const", bufs=1) as constp,
            tc.tile_pool(name="dram", bufs=1, space="DRAM") as dram,
            tc.tile_pool(name="proj", bufs=3) as projp,
            tc.tile_pool(name="psum", bufs=2, space="PSUM") as psump,
            tc.tile_pool(name="edge", bufs=2) as edgep,
            tc.tile_pool(name="fin", bufs=2) as finp,
        ):
            q_tab = dram.tile([NTN, D], F16)
            kv_slice = dram.tile([NTN, 2 * D], F16)
            kv_full = nc.dram_tensor(
                "kv_full", [NCORES * NTN, 2 * D], F16, addr_space="Shared")
            w_all = nc.dram_tensor(
                "w_all", [NCORES, 2, 128, 128], F16, addr_space="Shared")
            vn = dram.tile([NW * 128, D], F16)

            # ---- weights: AllGather the 8 shards, then lay out in SBUF ----
            wstage = dram.tile([2, 128, 128], F16)
            nc.sync.dma_start(wstage[:], wsh[:])
            nc.gpsimd.collective_compute(
                "AllGather",
                mybir.AluOpType.bypass,
                replica_groups=[list(range(NCORES))],
                ins=[wstage.opt()],
                outs=[w_all[:]],
            )
            # wpack_sb[p, j, c*128 + q] = w_all[c, j, p, q]
            wpack_sb = constp.tile([128, 2, NCORES * 128], F16)
            for j in (0, 1):
                nc.sync.dma_start(
                    wpack_sb[:, j, :].rearrange("p (c q) -> p c q", c=NCORES),
                    w_all[:, j].rearrange("c p q -> p c q"))
            wq_sb = wpack_sb[:, :, 0:D]
            wkv_sb = wpack_sb[:, :, D:3 * D]
            wa_sb = wpack_sb[:, :, 3 * D:4 * D]

            # ---- constants ----
            NTF = NPC // 128           # 19 full tiles
            NTAIL = NPC - NTF * 128    # 68 tail rows
            h8_sb = constp.tile([128, NTILES, D], F8)
            nc.vector.memset(h8_sb[:, NTF, :], 0.0)
            nc.sync.dma_start(
                h8_sb[:, 0:NTF, :],
                h8[0:NTF * 128].rearrange("(t p) d -> p t d", p=128))
            nc.sync.dma_start(h8_sb[0:NTAIL, NTF, :], h8[NTF * 128:NPC])
            h_sb = constp.tile([128, NTILES, D], F16)
            nc.vector.tensor_copy(h_sb[:], h8_sb[:])
            idx_sb = constp.tile([128, IDXL], I16)
            for g in range(8):
                nc.sync.dma_start(idx_sb[g * 16:(g + 1) * 16, :], idxp[:])
            sidx_sb = idx_sb[:, 0:NW * 128]
            qidx_sb = idx_sb[:, NW * 128:2 * NW * 128]
            vidx_sb = idx_sb[:, 2 * NW * 128:IDXL]
            colx_sb = constp.tile([128, NW * WCH], U8)
            nc.sync.dma_start(colx_sb[:], colx[:])
            iota_sb = constp.tile([128, WCH, 128], U8)
            nc.gpsimd.iota(
                iota_sb[:], [[0, WCH], [1, 128]], channel_multiplier=0,
                allow_small_or_imprecise_dtypes=True,
            )
            ident = constp.tile([128, 128], F16)
            nc.vector.memset(ident[:], 0.0)
            nc.gpsimd.affine_select(
                out=ident[:], in_=ident[:],
                compare_op=mybir.AluOpType.not_equal, fill=1.0,
                base=0, pattern=[[-1, 128]], channel_multiplier=1,
            )
            if use_bias or use_fbias:
                ones_sb = constp.tile([1, 128], F16)
                nc.vector.memset(ones_sb[:], 1.0)
            if use_bias:
                bqkv_sb = constp.tile([1, 3 * D], F16)
                nc.sync.dma_start(bqkv_sb[:], bqkv[:])
            if use_fbias:
                bfin_sb = constp.tile([1, D], F16)
                nc.sync.dma_start(bfin_sb[:], bfin[:])

            # ---- transpose h: hT_sb[:, j, node] = h[node, j*128+p] ----
            hT_sb = constp.tile([128, 2, NTN], F16)
            for nt in range(NTILES):
                for j in (0, 1):
                    pt = psump.tile([128, 128], F16, tag="pt")
                    nc.tensor.transpose(
                        pt[:], h_sb[:, nt, j * 128:(j + 1) * 128], ident[:])
                    nc.vector.tensor_copy(
                        hT_sb[:, j, nt * 128:(nt + 1) * 128], pt[:])

            # ---- projection phase ----
            for nt in range(NTILES):
                sl = slice(nt * 128, (nt + 1) * 128)
                pkv = psump.tile([128, 2 * D], F32, tag="pkv")
                for j in (0, 1):
                    nc.tensor.matmul(
                        pkv[:], hT_sb[:, j, sl], wkv_sb[:, j, :],
                        start=(j == 0), stop=(j == 1 and not use_bias),
                    )
                if use_bias:
                    nc.tensor.matmul(
                        pkv[:], ones_sb[:], bqkv_sb[:, D:3 * D],
                        start=False, stop=True)
                kv_sb = projp.tile([128, 2 * D], F16, tag="kv")
                nc.vector.tensor_copy(kv_sb[:], pkv[:])
                nc.sync.dma_start(kv_slice[sl, :], kv_sb[:])

                pq = psump.tile([128, D], F32, tag="pq")
                for j in (0, 1):
                    nc.tensor.matmul(
                        pq[:], hT_sb[:, j, sl], wq_sb[:, j, :],
                        start=(j == 0), stop=(j == 1 and not use_bias),
                    )
                if use_bias:
                    nc.tensor.matmul(
                        pq[:], ones_sb[:], bqkv_sb[:, 0:D],
                        start=False, stop=True)
                q_sb = projp.tile([128, D], F16, tag="q")
                nc.vector.tensor_copy(q_sb[:], pq[:])
                nc.sync.dma_start(q_tab[sl, :], q_sb[:])

            nc.gpsimd.collective_compute(
                "AllGather",
                mybir.AluOpType.bypass,
                replica_groups=[list(range(NCORES))],
                ins=[kv_slice.opt()],
                outs=[kv_full[:]],
            )

            # ---- edge phase ----
            for w in range(NW):
                csl = slice(w * 128, (w + 1) * 128)
                kvg = edgep.tile([128, WCH, 2 * D], F16, tag="kvg")
                nc.gpsimd.dma_gather(
                    kvg[:], kv_full[:], sidx_sb[:, csl],
                    num_idxs=WSLOTS, num_idxs_reg=WSLOTS, elem_size=2 * D,
                    single_packet=False,
                )
                qg = edgep.tile([128, WCH, D], F16, tag="qg")
                nc.gpsimd.dma_gather(
                    qg[:], q_tab[:], qidx_sb[:, csl],
                    num_idxs=WSLOTS, num_idxs_reg=WSLOTS, elem_size=D,
                    single_packet=False,
                )
                oa_sb = edgep.tile([128, WCH, 128], F16, tag="oa")
                nc.vector.tensor_tensor(
                    oa_sb[:],
                    colx_sb[:, w * WCH:(w + 1) * WCH].broadcast_to([128, WCH, 128]),
                    iota_sb[:],
                    mybir.AluOpType.is_equal,
                )

                prod = edgep.tile([128, WCH, D], F16, tag="prod")
                nc.vector.tensor_mul(prod[:], qg[:], kvg[:, :, 0:D])
                scores = edgep.tile([128, WCH, H], F32, tag="sc")
                nc.vector.tensor_reduce(
                    scores[:],
                    prod[:].rearrange("p c (h k) -> p c h k", h=H),
                    axis=mybir.AxisListType.X,
                    op=mybir.AluOpType.add,
                )
                msgz = edgep.tile([128, WCH, D + H], F16, tag="msgz")
                nc.scalar.activation(
                    msgz[:, :, D:D + H], scores[:], mybir.ActivationFunctionType.Exp
                )
                nc.vector.tensor_mul(
                    msgz[:, :, 0:D].rearrange("p c (h k) -> p c h k", h=H),
                    kvg[:, :, D:2 * D].rearrange("p c (h k) -> p c h k", h=H),
                    msgz[:, :, D:D + H].broadcast_to([128, WCH, H, DK]),
                )
                pw = psump.tile([128, D + H], F32, tag="pkv")
                for i in range(WCH):
                    nc.tensor.matmul(
                        pw[:], oa_sb[:, i, :], msgz[:, i, :],
                        start=(i == 0), stop=(i == WCH - 1),
                    )
                zr = finp.tile([128, H], F32, tag="zr")
                nc.vector.tensor_scalar_add(zr[:], pw[:, D:D + H], 1e-30)
                zrec = finp.tile([128, H], F32, tag="zrec")
                nc.vector.reciprocal(zrec[:], zr[:])
                vb = finp.tile([128, D], F16, tag="vb")
                nc.vector.tensor_mul(
                    vb[:].rearrange("p (h k) -> p h k", h=H),
                    pw[:, 0:D].rearrange("p (h k) -> p h k", h=H),
                    zrec[:].broadcast_to([128, H, DK]),
                )
                nc.sync.dma_start(vn[csl, :], vb[:])

            # ---- final phase ----
            tg = constp.tile([128, 2, NTN], F16)
            nc.gpsimd.dma_gather(
                tg[:], vn[:], vidx_sb[:],
                num_idxs=NTN, num_idxs_reg=NTN, elem_size=D, transpose=True,
                single_packet=False,
            )
            for nt in range(NTILES):
                sl = slice(nt * 128, (nt + 1) * 128)
                po = psump.tile([128, D], F32, tag="pq")
                for j in (0, 1):
                    nc.tensor.matmul(
                        po[:], tg[:, j, sl], wa_sb[:, j, :],
                        start=(j == 0), stop=(j == 1 and not use_fbias),
                    )
                if use_fbias:
                    nc.tensor.matmul(
                        po[:], ones_sb[:], bfin_sb[:], start=False, stop=True)
                ot = finp.tile([128, D], F8, tag="ot")
                nc.vector.tensor_copy(ot[:], po[:])
                if (nt + 1) * 128 <= NPC:
                    nc.sync.dma_start(outp[sl, :], ot[:])
                elif nt * 128 < NPC:
                    nc.sync.dma_start(outp[nt * 128:NPC, :], ot[0:NPC - nt * 128, :])

    nc.compile()
    return nc


def _make_runner(nc):
    install_neuronx_cc_hook()
    partition_name = nc.partition_id_tensor.name if nc.partition_id_tensor else None
    in_names, out_names, out_avals = [], [], []
    for alloc in nc.m.functions[0].allocations:
        if not isinstance(alloc, mybir.MemoryLocationSet):
            continue
        name = alloc.memorylocations[0].name
        if alloc.kind == "ExternalInput":
            if name != partition_name:
                in_names.append(name)
        elif alloc.kind == "ExternalOutput":
            out_names.append(name)
            out_avals.append(jax.core.ShapedArray(
                tuple(alloc.tensor_shape), mybir.dt.np(alloc.dtype)))
    n_params = len(in_names)
    bind_names = in_names + out_names
    if partition_name is not None:
        bind_names = bind_names + [partition_name]
    donate = tuple(range(n_params, n_params + len(out_names)))

    def _body(*args):
        operands = list(args)
        if partition_name is not None:
            operands.append(partition_id_tensor())
        outs = _bass_exec_p.bind(
            *operands,
            out_avals=tuple(out_avals),
            in_names=tuple(bind_names),
            out_names=tuple(out_names),
            lowering_input_output_aliases=(),
            sim_require_finite=True,
            sim_require_nnan=True,
            nc=nc,
        )
        return tuple(outs)

    mesh, zsh = _mesh_sh()
    in_specs = (P("core"),) * (n_params + len(out_names))
    out_specs = (P("core"),) * len(out_names)
    fn = jax.jit(
        shard_map(_body, mesh=mesh, in_specs=in_specs, out_specs=out_specs,
                  check_rep=False),
        donate_argnums=donate, keep_unused=True,
    )
    zeros_fn = jax.jit(
        lambda: tuple(
            jnp.zeros((NCORES * a.shape[0], *a.shape[1:]), a.dtype)
            for a in out_avals),
        out_shardings=(zsh,) * len(out_names) if len(out_names) > 1 else zsh,
    )

    import os, time as _t
    dbg = bool(os.environ.get("KERNEL_TIMING"))
    pool = _cf.ThreadPoolExecutor(NCORES)

    def run(globals_map, zeros=None):
        t0 = _t.perf_counter()
        args = [globals_map[name] for name in in_names]
        t1 = _t.perf_counter()
        if zeros is None:
            zeros = zeros_fn()
        if len(out_names) == 1 and not isinstance(zeros, tuple):
            zeros = (zeros,)
        t2 = _t.perf_counter()
        out_arrs = fn(*args, *zeros)
        t3 = _t.perf_counter()
        results = {}
        for i, name in enumerate(out_names):
            shards = sorted(out_arrs[i].addressable_shards,
                            key=lambda s: s.device.id)
            datas = [s.data for s in shards]
            for d in datas:
                try:
                    d.copy_to_host_async()
                except AttributeError:
                    break
            parts = list(pool.map(np.asarray, datas))
            results[name] = parts
        t4 = _t.perf_counter()
        if dbg:
            print(f"[run] gather_args={t1-t0:.3f} zeros={t2-t1:.3f} "
                  f"dispatch={t3-t2:.3f} fetch={t4-t3:.3f}", flush=True)
        return results

    run.fn = fn
    run.zeros_fn = zeros_fn
    run.in_names = in_names
    run.out_names = out_names
    return run


def _wrap16(v):
    """[L] int array -> [16, L//16] wrapped int16: tile[p, s] = v[s*16+p]."""
    L = v.shape[0]
    return np.ascontiguousarray(v.reshape(L // 16, 16).T.astype(np.int16))


def _wrap16_win(v):
    """[NW, WSLOTS] -> [16, NW*128]: per-window wrapped layout."""
    NW = v.shape[0]
    w = v.reshape(NW, WSLOTS // 16, 16).transpose(2, 0, 1)
    return np.ascontiguousarray(w.reshape(16, NW * (WSLOTS // 16)).astype(np.int16))


def kernel(h, src, dst, Wk, bk, Wq, bq, Wv, bv, Wa, ba, rel_att, rel_msg, rel_pri, skip):
    global LAST_RESULTS, LAST_EXEC_NS
    h = np.asarray(h, np.float32)
    src = np.asarray(src, np.int32)
    dst = np.asarray(dst, np.int32)

    # ---- fold weights on host ----
    scale = (np.asarray(rel_pri, np.float32) / math.sqrt(DK)).astype(np.float32)
    WqT = np.asarray(Wq, np.float32).T.reshape(D, H, DK)
    Wq_eff = (WqT * scale[None, :, None]).reshape(D, D)
    bq_eff = (np.asarray(bq, np.float32).reshape(H, DK) * scale[:, None]).reshape(D)
    WkT = np.asarray(Wk, np.float32).T.reshape(D, H, DK)
    Wk_eff = np.einsum("dhk,hke->dhe", WkT, np.asarray(rel_att, np.float32)).reshape(D, D)
    bk_eff = np.einsum("hk,hke->he", np.asarray(bk, np.float32).reshape(H, DK),
                       np.asarray(rel_att, np.float32)).reshape(D)
    WvT = np.asarray(Wv, np.float32).T.reshape(D, H, DK)
    Wv_eff = np.einsum("dhk,hke->dhe", WvT, np.asarray(rel_msg, np.float32)).reshape(D, D)
    bv_eff = np.einsum("hk,hke->he", np.asarray(bv, np.float32).reshape(H, DK),
                       np.asarray(rel_msg, np.float32)).reshape(D)
    alpha = float(1.0 / (1.0 + math.exp(-float(np.asarray(skip)))))
    # h is uploaded fp8, pre-scaled by s8 to fit e4m3 range; W{q,k,v} absorb
    # 1/s8 (scores and v are then exact w.r.t. the scaling). The skip term
    # (1-alpha)*h is added on the host from the fp32 h, so the device output
    # is just alpha*trans, scaled by OSC to sit in e4m3's normal range.
    hmax = float(np.abs(h).max()) if h.size else 1.0
    s8 = 1.0 if hmax <= 192.0 else 128.0 / hmax
    OSC = 32.0
    Wq_dev = Wq_eff / s8
    Wkv_dev = np.concatenate([Wk_eff, Wv_eff], axis=1) / s8   # [256, 512]
    bkv_eff = np.concatenate([bk_eff, bv_eff])                # [512]
    Wa_dev = (OSC * alpha * np.asarray(Wa, np.float32).T)     # [256, 256]
    ba_fin = OSC * alpha * np.asarray(ba, np.float32)
    use_bias = bool(np.any(bq_eff) or np.any(bkv_eff))
    use_fbias = bool(np.any(ba_fin))

    # ---- start the big uploads NOW: h8 (2/3 of the upload bytes) and the
    # weight shards stream over the tunnel asynchronously while the host
    # does the edge preprocessing below (transfer/compute overlap) ----
    f16 = np.float16
    f8np = mybir.dt.np(F8)
    _, csh = _mesh_sh()
    h8_g = jax.device_put(np.ascontiguousarray((s8 * h).astype(f8np)), csh)
    wpack = np.concatenate([Wq_dev, Wkv_dev, Wa_dev], axis=1)  # [256, 1024]
    wpack = np.ascontiguousarray(
        wpack.reshape(2, 128, NCORES, 128).transpose(2, 0, 1, 3)
        .astype(f16).reshape(NCORES * 2, 128, 128))
    wsh_g = jax.device_put(wpack, csh)

    # ---- edge preprocessing ----
    order = np.argsort(dst, kind="stable")
    dsts = dst[order]
    srcs = src[order]
    core_of = dsts // NPC
    core_starts = np.searchsorted(core_of, np.arange(NCORES + 1))
    deg = np.bincount(dst, minlength=N)

    # window packing per core: LPT bin packing of nodes into windows of
    # <=WSPAN nodes and <=WSLOTS edge slots (largest-degree first into the
    # least-loaded window). Windows are arbitrary node sets; each node gets
    # (win_id, col) used for the one-hot column, q-row index, and its row
    # w*128+col in the vn buffer. Retries with one more window on overflow.
    def _lpt_pack(degs, nb):
        order_d = np.argsort(-degs, kind="stable")
        used = np.zeros(nb, np.int64)
        cnt = np.zeros(nb, np.int64)
        win_id = np.empty(NPC, np.int64)
        col_of = np.empty(NPC, np.int64)
        for node in order_d:
            d = int(degs[node])
            cand = np.where(cnt < WSPAN)[0]
            b = cand[np.argmin(used[cand])]
            if used[b] + d > WSLOTS:
                return None
            win_id[node] = b
            col_of[node] = cnt[b]
            used[b] += d
            cnt[b] += 1
        return win_id, col_of

    core_meta = []  # per core: (win_id[NPC], col_of[NPC], n_windows)
    NW_max = 0
    for c in range(NCORES):
        degs = deg[c * NPC:(c + 1) * NPC]
        assert degs.max(initial=0) <= WSLOTS, "node degree exceeds window capacity"
        nb = max(int(np.ceil(degs.sum() / WSLOTS)),
                 int(np.ceil(NPC / WSPAN)))
        while True:
            r = _lpt_pack(degs, nb)
            if r is not None:
                break
            nb += 1
        core_meta.append((r[0], r[1], nb))
        NW_max = max(NW_max, nb)
    NW = NW_max

    key = (NW, use_bias, use_fbias)
    if key not in _cache:
        nc = _build(NW, use_bias, use_fbias)
        _cache[key] = (nc, _make_runner(nc))
    nc, run = _cache[key]

    # ---- per-core index tables (pass 1: idxp, so its upload can start
    # streaming while pass 2 builds colx below) ----
    idxp_parts = []
    colx_stash = []
    if use_bias:
        bqkv_in = np.concatenate([bq_eff, bkv_eff]).reshape(1, 3 * D).astype(f16)
    if use_fbias:
        bfin_in = ba_fin.reshape(1, D).astype(f16)

    for c in range(NCORES):
        n0 = c * NPC
        e0, e1 = core_starts[c], core_starts[c + 1]
        ed = dsts[e0:e1] - n0         # local dst
        es = srcs[e0:e1]              # global src
        win_id, col_of, _nw = core_meta[c]
        wid = win_id[ed]
        # slot assignment: within window, sort edges by src kv row for locality
        es_row = (es // NPC) * NTN + (es % NPC)
        sort2 = np.lexsort((es_row, wid))
        ed = ed[sort2]
        es_row = es_row[sort2]
        wid = wid[sort2]
        # rank within window
        wcounts = np.bincount(wid, minlength=NW)
        woff = np.zeros(NW + 1, np.int64)
        np.cumsum(wcounts, out=woff[1:])
        rank = np.arange(e1 - e0) - woff[wid]
        slot = wid * WSLOTS + rank    # global slot in [0, NW*WSLOTS)

        src_slots = np.zeros((NW, WSLOTS), np.int64)
        q_slots = np.zeros((NW, WSLOTS), np.int64)
        src_slots.reshape(-1)[slot] = es_row
        q_slots.reshape(-1)[slot] = ed

        # vrow: local node -> its row in vn
        vrow = np.zeros(NTN, np.int64)
        vrow[:NPC] = win_id * 128 + col_of

        idxp_parts.append(np.concatenate(
            [_wrap16_win(src_slots), _wrap16_win(q_slots), _wrap16(vrow)], axis=1))
        colx_stash.append((slot, col_of[ed]))

    globals_map = {
        "h8": h8_g,
        "wsh": wsh_g,
        "idxp": jax.device_put(np.concatenate(idxp_parts, axis=0), csh),
    }

    # ---- pass 2: colx one-hot columns build while idxp streams ----
    colx_parts = []
    for slot, colv in colx_stash:
        # one-hot column per slot: [128, NW*WCH]; slot s=(w, c*128+p) ->
        # colx[p, w*WCH+c]; 255 for empty slots (matches no iota value)
        col_slots = np.full((NW, WSLOTS), 255, np.int64)
        col_slots.reshape(-1)[slot] = colv
        colx_parts.append(np.ascontiguousarray(
            col_slots.reshape(NW, WCH, 128).transpose(2, 0, 1)
            .reshape(128, NW * WCH).astype(np.uint8)))
    globals_map["colx"] = jax.device_put(np.concatenate(colx_parts, axis=0), csh)
    if use_bias:
        globals_map["bqkv"] = jax.device_put(
            np.concatenate([bqkv_in] * NCORES, axis=0), csh)
    if use_fbias:
        globals_map["bfin"] = jax.device_put(
            np.concatenate([bfin_in] * NCORES, axis=0), csh)
    # donated output buffers materialize on device and the host skip term
    # computes while the index tables stream to the devices
    zeros = run.zeros_fn()
    skip_part = (1.0 - alpha) * h

    global LAST_INMAPS
    LAST_INMAPS = globals_map
    import time as _time
    _t0 = _time.perf_counter()
    res = run(globals_map, zeros)
    LAST_RESULTS = res
    LAST_EXEC_NS = int((_time.perf_counter() - _t0) * 1e9)

    out = np.concatenate(
        [res["out"][c] for c in range(NCORES)], axis=0).astype(np.float32)
    out *= 1.0 / OSC
    out += skip_part
    return out
